# revision 13
# baseline (speedup 1.0000x reference)
"""HardNegativeInfoNCELoss on 8 Trainium2 NeuronCores.

Strategy (v4, exp-accumulate scan):
  * Host: L2-normalize anchor/positive/negative_pool (fp32), scale by 64 and
    quantize to fp8 e4m3 with the K=256 contraction packed as 2 k-tiles
    (DoubleRow).  Pool columns sharded across 8 cores (M/8 = 32768).
  * Device (SPMD, per core): stream the pool shard chunk-by-chunk (2048
    cols).  Per (128-anchor b-tile, 1024-col half-chunk) run 2 fp8 DR
    matmuls into a [128,1024] PSUM tile (4-deep rotation, PE ~216ns/MM).
    Each PSUM tile is consumed in ONE pass by one of two engines:
      - ScalarE: activation(Exp, scale=a, bias=-a*C) with accum_out ->
        acc = sum_j exp(a*(s_j - C)); the host recovers the window max as
        C + ln(acc)/a (exact to ~0.3 scaled units since the sum is
        max-dominated at a=0.11).  W=1024 windows, ~1.33us/tile.
      - VectorE: windowed tensor_reduce max [128,2,512] -> [128,2],
        W=512 windows, ~1.2us/tile.
    Both engines run ~1 elem/cycle; the scan is the critical path with the
    matmul stream (6.9us/chunk) hidden under it.  ACT handles 120 tiles,
    DVE 136 (8 h=0 tiles flipped to DVE on odd chunks at b=7).
  * Host: candidates = ACT lse-maxes + DVE window maxes (~400 per row);
    exact top-10 per anchor, exact fp32 positive logit, InfoNCE loss.

  Window-collision safety: a window of W cols keeps only its max, losing a
  true top-10 member only when two land in one window (~9-18% of rows for
  W=512-1024); the lost member is replaced by rank 11 shifting the loss
  ~1e-4 relative.  Host-validated end to end: rel err 6.2e-4 vs fp32
  reference (gate 2e-2).
"""

import os
import sys

import numpy as np


def _ensure_concourse():
    try:
        import concourse  # noqa: F401
        return
    except ImportError:
        pass
    for p in ("/opt/trn_rl_repo", "/root/.axon_site/_ro/trn_rl_repo"):
        if os.path.isdir(os.path.join(p, "concourse")):
            sys.path.insert(0, p)
            return


_ensure_concourse()

N_CORES = 8
B = 1024
D = 256
M = 262144
M_SHARD = M // N_CORES  # 32768
CHUNK = 2048
N_CHUNKS = M_SHARD // CHUNK  # 16
NB = B // 128  # 8 anchor tiles
SCALE = 64.0
TEMPERATURE = 0.07
NUM_HARD_NEGATIVES = 10
EPS = 1e-12
ALPHA = 0.11
C_SHIFT = 1100.0

# tile (c, b, h): h=0 -> ACT exp-acc (W=1024), h=1 -> DVE reduce (2x W=512),
# except flipped tiles (c odd, b == 7, h == 0) which go to DVE too,
# balancing ACT 120 : DVE 136 tiles per core (ACT ~1330ns/tile vs DVE ~1175).
N_FLIP = 8
N_ACC = N_CHUNKS * NB              # 128 cols (8 never written -> 0)
N_RED = N_CHUNKS * NB * 2 + 2 * N_FLIP  # 272 cols

_program = None


def _is_flip(c, b):
    return (c % 2 == 1) and (b == 7)


def _build_program():
    import concourse.bacc as bacc
    import concourse.mybir as mybir
    from concourse.tile import TileContext

    nc = bacc.Bacc(
        "TRN2", target_bir_lowering=False, debug=False, num_devices=N_CORES
    )
    f32 = mybir.dt.float32
    fp8 = mybir.dt.float8e4
    DR = mybir.MatmulPerfMode.DoubleRow
    Exp = mybir.ActivationFunctionType.Exp
    Max = mybir.AluOpType.max
    X = mybir.AxisListType.X

    # AT8[p, t*1024 + m] = a8[m, t*128 + p];  PT8[p, c*4096 + t*2048 + j]
    AT8 = nc.dram_tensor("AT8", [128, 2 * B], fp8, kind="ExternalInput")
    PT8 = nc.dram_tensor("PT8", [128, 2 * M_SHARD], fp8, kind="ExternalInput")
    ACC = nc.dram_tensor("ACC", [128, N_ACC], f32, kind="ExternalOutput")
    RED = nc.dram_tensor("RED", [128, N_RED], f32, kind="ExternalOutput")

    with TileContext(nc) as tc:
        with (
            tc.tile_pool(name="const", bufs=1) as cpool,
            tc.tile_pool(name="stream", bufs=2) as spool,
            tc.tile_pool(name="psum", bufs=4, space="PSUM") as ppool,
        ):
            at8 = cpool.tile([128, 2 * B], fp8)
            nc.sync.dma_start(out=at8, in_=AT8[:, :])
            a8v = at8[:, :].rearrange("p (t m) -> p t m", t=2)  # [128,2,1024]

            biasap = cpool.tile([128, 1], f32)
            nc.vector.memset(biasap, -ALPHA * C_SHIFT)

            accb = cpool.tile([128, N_ACC], f32)
            nc.vector.memset(accb, 0.0)
            redb = cpool.tile([128, N_RED], f32)

            scratch = cpool.tile([128, 1024], f32)

            nflip = 0
            for c in range(N_CHUNKS):
                pt = spool.tile([128, 2 * CHUNK], fp8, tag="pt", name="pt")
                ptv = pt[:, :].rearrange("p (t j) -> p t j", t=2)
                if c == 0:
                    # split the first chunk's fetch by j-halves so the h=1
                    # matmuls can start ~2us before the full 512KB lands
                    p8v = PT8[:, 0: 2 * CHUNK].rearrange(
                        "p (t j) -> p t j", t=2)
                    nc.sync.dma_start(
                        out=ptv[:, :, 1024:2048], in_=p8v[:, :, 1024:2048])
                    nc.sync.dma_start(
                        out=ptv[:, :, 0:1024], in_=p8v[:, :, 0:1024])
                else:
                    nc.sync.dma_start(
                        out=pt, in_=PT8[:, c * 2 * CHUNK: (c + 1) * 2 * CHUNK]
                    )

                for b in range(NB):
                    a_sl = a8v[:, :, b * 128: (b + 1) * 128]
                    for h in (1, 0):
                        ps = ppool.tile([128, 1024], f32, tag="ps", name="ps")
                        for n in range(2):
                            col = h * 1024 + n * 512
                            nc.tensor.matmul(
                                ps[:, n * 512: (n + 1) * 512],
                                a_sl,
                                ptv[:, :, col: col + 512],
                                start=True, stop=True, perf_mode=DR,
                            )
                        ti = c * NB + b
                        if h == 0 and not _is_flip(c, b):
                            nc.scalar.activation(
                                out=scratch, in_=ps, func=Exp,
                                bias=biasap[:, :], scale=ALPHA,
                                accum_out=accb[:, ti: ti + 1],
                            )
                        else:
                            if h == 1:
                                o = ti * 2
                            else:
                                o = N_CHUNKS * NB * 2 + 2 * nflip
                                nflip += 1
                            psw = ps[:, :].rearrange("p (g s) -> p g s", g=2)
                            nc.vector.tensor_reduce(
                                out=redb[:, o: o + 2], in_=psw, axis=X, op=Max,
                            )
                if c == 7:
                    # first half of RED is complete; drain it early
                    nc.sync.dma_start(out=RED[:, 0:128], in_=redb[:, 0:128])
            nc.sync.dma_start(out=ACC[:, :], in_=accb)
            nc.sync.dma_start(out=RED[:, 128:N_RED], in_=redb[:, 128:N_RED])
    nc.compile()
    return nc


def _get_program():
    global _program
    if _program is None:
        _program = _build_program()
    return _program


def _normalize_rows(x):
    n = np.sqrt((x.astype(np.float32) ** 2).sum(axis=-1, keepdims=True))
    return x / np.maximum(n, EPS)


def run_device(anchor, negative_pool, trace=False, tmpdir=None):
    """Run the SPMD device program; returns (per-core (ACC, RED) list, results)."""
    from concourse.bass_utils import run_bass_kernel_spmd

    import ml_dtypes

    fp8 = ml_dtypes.float8_e4m3
    a = _normalize_rows(np.asarray(anchor, dtype=np.float32)) * SCALE
    n = _normalize_rows(np.asarray(negative_pool, dtype=np.float32)) * SCALE
    a8 = a.astype(fp8)  # [B, 256]
    n8 = n.astype(fp8)  # [M, 256]

    # AT8[p, t, m] = a8[m, t*128+p]
    at8 = np.ascontiguousarray(
        a8.reshape(B, 2, 128).transpose(2, 1, 0)
    ).reshape(128, 2 * B)
    in_maps = []
    for core in range(N_CORES):
        sh = n8[core * M_SHARD: (core + 1) * M_SHARD]  # [32768, 256]
        # PT8[p, c, t, j] = sh[c*2048 + j, t*128 + p]
        pt8 = np.ascontiguousarray(
            sh.reshape(N_CHUNKS, CHUNK, 2, 128).transpose(3, 0, 2, 1)
        ).reshape(128, 2 * M_SHARD)
        in_maps.append({"AT8": at8, "PT8": pt8})
    nc = _get_program()
    res = run_bass_kernel_spmd(
        nc, in_maps, core_ids=list(range(N_CORES)), trace=trace, tmpdir=tmpdir
    )
    outs = [(res.results[c]["ACC"], res.results[c]["RED"])
            for c in range(N_CORES)]
    return outs, res


def merge_loss(anchor, positive, outs):
    a = _normalize_rows(np.asarray(anchor, dtype=np.float32))
    p = _normalize_rows(np.asarray(positive, dtype=np.float32))
    pos_sim = (a * p).sum(axis=-1, dtype=np.float32) / TEMPERATURE  # [B]

    inv = 1.0 / (SCALE * SCALE)
    parts = []  # per-core candidate arrays [B, ncand]
    for acc, red in outs:
        acc = np.asarray(acc, dtype=np.float32)  # [128, 128] cols = c*8+b
        red = np.asarray(red, dtype=np.float32)  # [128, 272]
        with np.errstate(divide="ignore"):
            lse = np.log(acc) / ALPHA + C_SHIFT  # -inf where acc == 0
        # candidates for row b*128+p live in acc[p, c*8+b], red[p, (c*8+b)*2+k]
        av = lse.reshape(128, N_CHUNKS, NB)          # [p, c, b]
        rv = red[:, : N_CHUNKS * NB * 2].reshape(128, N_CHUNKS, NB, 2)
        cand_b = []  # [b][128, ncand]
        for b in range(NB):
            cols = [av[:, :, b], rv[:, :, b, 0], rv[:, :, b, 1]]
            if b == 7:
                fl = red[:, N_CHUNKS * NB * 2:]      # [128, 16]
                cols.append(fl)
            cand_b.append(np.concatenate(cols, axis=1))
        w = max(x.shape[1] for x in cand_b)
        cand_b = [
            np.pad(x, ((0, 0), (0, w - x.shape[1])), constant_values=-np.inf)
            for x in cand_b
        ]
        parts.append(np.stack(cand_b, 0).reshape(B, -1))
    cand = np.concatenate(parts, axis=1) * inv / TEMPERATURE  # [B, ncand]
    cand = np.nan_to_num(cand, nan=-np.inf, posinf=-np.inf, neginf=-np.inf)

    k = NUM_HARD_NEGATIVES
    part = np.partition(cand, cand.shape[1] - k, axis=1)[:, -k:]
    hard = np.sort(part, axis=1)[:, ::-1]

    logits = np.concatenate([pos_sim[:, None], hard], axis=1).astype(np.float64)
    mx = logits.max(axis=1, keepdims=True)
    lse = mx[:, 0] + np.log(np.exp(logits - mx).sum(axis=1))
    loss = -(logits[:, 0] - lse).mean()
    return np.float32(loss)


def kernel(anchor, positive, negative_pool):
    outs, _ = run_device(anchor, negative_pool)
    return np.asarray(merge_loss(anchor, positive, outs), dtype=np.float32)


# revision 14
# speedup vs baseline: 1.0267x; 1.0267x over previous
"""HardNegativeInfoNCELoss on 8 Trainium2 NeuronCores.

Strategy (v4, exp-accumulate scan):
  * Host: L2-normalize anchor/positive/negative_pool (fp32), scale by 64 and
    quantize to fp8 e4m3 with the K=256 contraction packed as 2 k-tiles
    (DoubleRow).  Pool columns sharded across 8 cores (M/8 = 32768).
  * Device (SPMD, per core): stream the pool shard chunk-by-chunk (2048
    cols).  Per (128-anchor b-tile, 1024-col half-chunk) run 2 fp8 DR
    matmuls into a [128,1024] PSUM tile (4-deep rotation, PE ~216ns/MM).
    Each PSUM tile is consumed in ONE pass by one of two engines:
      - ScalarE: activation(Exp, scale=a, bias=-a*C) with accum_out ->
        acc = sum_j exp(a*(s_j - C)); the host recovers the window max as
        C + ln(acc)/a (exact to ~0.3 scaled units since the sum is
        max-dominated at a=0.11).  W=1024 windows, ~1.33us/tile.
      - VectorE: windowed tensor_reduce max [128,2,512] -> [128,2],
        W=512 windows, ~1.2us/tile.
    Both engines run ~1 elem/cycle; the scan is the critical path with the
    matmul stream (6.9us/chunk) hidden under it.  ACT handles 120 tiles,
    DVE 136 (8 h=0 tiles flipped to DVE on odd chunks at b=7).
  * Host: candidates = ACT lse-maxes + DVE window maxes (~400 per row);
    exact top-10 per anchor, exact fp32 positive logit, InfoNCE loss.

  Window-collision safety: a window of W cols keeps only its max, losing a
  true top-10 member only when two land in one window (~9-18% of rows for
  W=512-1024); the lost member is replaced by rank 11 shifting the loss
  ~1e-4 relative.  Host-validated end to end: rel err 6.2e-4 vs fp32
  reference (gate 2e-2).
"""

import os
import sys

import numpy as np


def _ensure_concourse():
    try:
        import concourse  # noqa: F401
        return
    except ImportError:
        pass
    for p in ("/opt/trn_rl_repo", "/root/.axon_site/_ro/trn_rl_repo"):
        if os.path.isdir(os.path.join(p, "concourse")):
            sys.path.insert(0, p)
            return


_ensure_concourse()

N_CORES = 8
B = 1024
D = 256
M = 262144
M_SHARD = M // N_CORES  # 32768
CHUNK = 2048
N_CHUNKS = M_SHARD // CHUNK  # 16
NB = B // 128  # 8 anchor tiles
SCALE = 64.0
TEMPERATURE = 0.07
NUM_HARD_NEGATIVES = 10
EPS = 1e-12
ALPHA = 0.11
C_SHIFT = 1100.0

# tile (c, b, h): h=0 -> ACT exp-acc (W=1024), h=1 -> DVE reduce (2x W=512),
# except flipped tiles (c odd, b == 7, h == 0) which go to DVE too,
# balancing ACT 120 : DVE 136 tiles per core (ACT ~1330ns/tile vs DVE ~1175).
N_FLIP = 8
N_ACC = N_CHUNKS * NB              # 128 cols (8 never written -> 0)
N_RED = N_CHUNKS * NB * 2 + 2 * N_FLIP  # 272 cols

_program = None


def _is_flip(c, b):
    return (c % 2 == 1) and (b == 7)


def _build_program():
    import concourse.bacc as bacc
    import concourse.mybir as mybir
    from concourse.tile import TileContext

    nc = bacc.Bacc(
        "TRN2", target_bir_lowering=False, debug=False, num_devices=N_CORES
    )
    f32 = mybir.dt.float32
    fp8 = mybir.dt.float8e4
    DR = mybir.MatmulPerfMode.DoubleRow
    Exp = mybir.ActivationFunctionType.Exp
    Max = mybir.AluOpType.max
    X = mybir.AxisListType.X

    # AT8[p, t*1024 + m] = a8[m, t*128 + p];  PT8[p, c*4096 + t*2048 + j]
    AT8 = nc.dram_tensor("AT8", [128, 2 * B], fp8, kind="ExternalInput")
    PT8 = nc.dram_tensor("PT8", [128, 2 * M_SHARD], fp8, kind="ExternalInput")
    ACC = nc.dram_tensor("ACC", [128, N_ACC], f32, kind="ExternalOutput")
    RED = nc.dram_tensor("RED", [128, N_RED], f32, kind="ExternalOutput")

    with TileContext(nc) as tc:
        with (
            tc.tile_pool(name="const", bufs=1) as cpool,
            tc.tile_pool(name="stream", bufs=2) as spool,
            tc.tile_pool(name="psum_a", bufs=2, space="PSUM") as apool,
            tc.tile_pool(name="psum_v", bufs=2, space="PSUM") as vpool,
        ):
            at8 = cpool.tile([128, 2 * B], fp8)
            nc.sync.dma_start(out=at8, in_=AT8[:, :])
            a8v = at8[:, :].rearrange("p (t m) -> p t m", t=2)  # [128,2,1024]

            biasap = cpool.tile([128, 1], f32)
            nc.vector.memset(biasap, -ALPHA * C_SHIFT)

            accb = cpool.tile([128, N_ACC], f32)
            nc.vector.memset(accb, 0.0)
            redb = cpool.tile([128, N_RED], f32)

            scratch = cpool.tile([128, 1024], f32)

            nflip = 0
            for c in range(N_CHUNKS):
                pt = spool.tile([128, 2 * CHUNK], fp8, tag="pt", name="pt")
                ptv = pt[:, :].rearrange("p (t j) -> p t j", t=2)
                if c == 0:
                    # split the first chunk's fetch by j-halves so the h=1
                    # matmuls can start ~2us before the full 512KB lands
                    p8v = PT8[:, 0: 2 * CHUNK].rearrange(
                        "p (t j) -> p t j", t=2)
                    nc.sync.dma_start(
                        out=ptv[:, :, 1024:2048], in_=p8v[:, :, 1024:2048])
                    nc.sync.dma_start(
                        out=ptv[:, :, 0:1024], in_=p8v[:, :, 0:1024])
                else:
                    nc.sync.dma_start(
                        out=pt, in_=PT8[:, c * 2 * CHUNK: (c + 1) * 2 * CHUNK]
                    )

                for b in range(NB):
                    a_sl = a8v[:, :, b * 128: (b + 1) * 128]
                    for h in (1, 0):
                        is_act = h == 0 and not _is_flip(c, b)
                        pool = apool if is_act else vpool
                        ps = pool.tile([128, 1024], f32, tag="ps", name="ps")
                        for n in range(2):
                            col = h * 1024 + n * 512
                            nc.tensor.matmul(
                                ps[:, n * 512: (n + 1) * 512],
                                a_sl,
                                ptv[:, :, col: col + 512],
                                start=True, stop=True, perf_mode=DR,
                            )
                        ti = c * NB + b
                        if is_act:
                            nc.scalar.activation(
                                out=scratch, in_=ps, func=Exp,
                                bias=biasap[:, :], scale=ALPHA,
                                accum_out=accb[:, ti: ti + 1],
                            )
                        else:
                            if h == 1:
                                o = ti * 2
                            else:
                                o = N_CHUNKS * NB * 2 + 2 * nflip
                                nflip += 1
                            psw = ps[:, :].rearrange("p (g s) -> p g s", g=2)
                            nc.vector.tensor_reduce(
                                out=redb[:, o: o + 2], in_=psw, axis=X, op=Max,
                            )
                if c == 7:
                    # first half of RED is complete; drain it early
                    nc.sync.dma_start(out=RED[:, 0:128], in_=redb[:, 0:128])
            nc.sync.dma_start(out=ACC[:, :], in_=accb)
            nc.sync.dma_start(out=RED[:, 128:N_RED], in_=redb[:, 128:N_RED])
    nc.compile()
    return nc


def _get_program():
    global _program
    if _program is None:
        _program = _build_program()
    return _program


def _normalize_rows(x):
    n = np.sqrt((x.astype(np.float32) ** 2).sum(axis=-1, keepdims=True))
    return x / np.maximum(n, EPS)


def run_device(anchor, negative_pool, trace=False, tmpdir=None):
    """Run the SPMD device program; returns (per-core (ACC, RED) list, results)."""
    from concourse.bass_utils import run_bass_kernel_spmd

    import ml_dtypes

    fp8 = ml_dtypes.float8_e4m3
    a = _normalize_rows(np.asarray(anchor, dtype=np.float32)) * SCALE
    n = _normalize_rows(np.asarray(negative_pool, dtype=np.float32)) * SCALE
    a8 = a.astype(fp8)  # [B, 256]
    n8 = n.astype(fp8)  # [M, 256]

    # AT8[p, t, m] = a8[m, t*128+p]
    at8 = np.ascontiguousarray(
        a8.reshape(B, 2, 128).transpose(2, 1, 0)
    ).reshape(128, 2 * B)
    in_maps = []
    for core in range(N_CORES):
        sh = n8[core * M_SHARD: (core + 1) * M_SHARD]  # [32768, 256]
        # PT8[p, c, t, j] = sh[c*2048 + j, t*128 + p]
        pt8 = np.ascontiguousarray(
            sh.reshape(N_CHUNKS, CHUNK, 2, 128).transpose(3, 0, 2, 1)
        ).reshape(128, 2 * M_SHARD)
        in_maps.append({"AT8": at8, "PT8": pt8})
    nc = _get_program()
    res = run_bass_kernel_spmd(
        nc, in_maps, core_ids=list(range(N_CORES)), trace=trace, tmpdir=tmpdir
    )
    outs = [(res.results[c]["ACC"], res.results[c]["RED"])
            for c in range(N_CORES)]
    return outs, res


def merge_loss(anchor, positive, outs):
    a = _normalize_rows(np.asarray(anchor, dtype=np.float32))
    p = _normalize_rows(np.asarray(positive, dtype=np.float32))
    pos_sim = (a * p).sum(axis=-1, dtype=np.float32) / TEMPERATURE  # [B]

    inv = 1.0 / (SCALE * SCALE)
    parts = []  # per-core candidate arrays [B, ncand]
    for acc, red in outs:
        acc = np.asarray(acc, dtype=np.float32)  # [128, 128] cols = c*8+b
        red = np.asarray(red, dtype=np.float32)  # [128, 272]
        with np.errstate(divide="ignore"):
            lse = np.log(acc) / ALPHA + C_SHIFT  # -inf where acc == 0
        # candidates for row b*128+p live in acc[p, c*8+b], red[p, (c*8+b)*2+k]
        av = lse.reshape(128, N_CHUNKS, NB)          # [p, c, b]
        rv = red[:, : N_CHUNKS * NB * 2].reshape(128, N_CHUNKS, NB, 2)
        cand_b = []  # [b][128, ncand]
        for b in range(NB):
            cols = [av[:, :, b], rv[:, :, b, 0], rv[:, :, b, 1]]
            if b == 7:
                fl = red[:, N_CHUNKS * NB * 2:]      # [128, 16]
                cols.append(fl)
            cand_b.append(np.concatenate(cols, axis=1))
        w = max(x.shape[1] for x in cand_b)
        cand_b = [
            np.pad(x, ((0, 0), (0, w - x.shape[1])), constant_values=-np.inf)
            for x in cand_b
        ]
        parts.append(np.stack(cand_b, 0).reshape(B, -1))
    cand = np.concatenate(parts, axis=1) * inv / TEMPERATURE  # [B, ncand]
    cand = np.nan_to_num(cand, nan=-np.inf, posinf=-np.inf, neginf=-np.inf)

    k = NUM_HARD_NEGATIVES
    part = np.partition(cand, cand.shape[1] - k, axis=1)[:, -k:]
    hard = np.sort(part, axis=1)[:, ::-1]

    logits = np.concatenate([pos_sim[:, None], hard], axis=1).astype(np.float64)
    mx = logits.max(axis=1, keepdims=True)
    lse = mx[:, 0] + np.log(np.exp(logits - mx).sum(axis=1))
    loss = -(logits[:, 0] - lse).mean()
    return np.float32(loss)


def kernel(anchor, positive, negative_pool):
    outs, _ = run_device(anchor, negative_pool)
    return np.asarray(merge_loss(anchor, positive, outs), dtype=np.float32)
